# revision 9
# baseline (speedup 1.0000x reference)
"""Cross-attention kernel for Trainium2 (8 NeuronCores, SPMD).

Problem: out = x_a + gamma * attn_out where
  q = Wq @ xa + bq   [B, N, CK]     (1x1 conv == per-pixel linear)
  k = Wk @ xb + bk   [B, CK, N]
  v = Wv @ xb + bv   [B, N, C]
  attn_out = softmax(q @ k, axis=-1) @ v   (transposed back to [B, C, H, W])
with B=4, C=256, CK=32, N=64*64=4096.

Sharding: 8 cores = (batch b, n-half) pairs. Each core computes q for its
2048 rows, full k/v for its batch (replicated work within a batch pair),
and its 2048xN attention rows locally. No cross-core communication.

On-core dataflow (all matmuls in float32r, 1 PE cycle/row):
  qT [CK, n]  = WqT.T @ xa       kmat [CK, m] = WkT.T @ b
  both zero-padded to K=128 rows (K=32 matmuls run ~2x slower than K=128
  on the PE, measured) -> ST tile [m=128, n=512x2] = kmat_pad.T @ qT_pad
  expST = Exp(ST) on ScalarE (softmax without max-subtraction: logits are
  bounded ~|47| here, exp stays finite in fp32)
  out_aug [n, 258] += expST.T @ v_aug  where v_aug = [v | ones | pad]
  => column 256 accumulates the softmax denominator for free.
  finalize: scale rows by gamma/rowsum, PE-transpose to [c, n], add x_a,
  DMA out in [C, N] layout.
"""
import numpy as np

import concourse.bass as bass
import concourse.mybir as mybir
import concourse.tile as tile
from concourse import bacc, bass_utils
from concourse.masks import make_identity

F32 = mybir.dt.float32
F32R = mybir.dt.float32r
EXP = mybir.ActivationFunctionType.Exp

B, C, H, W = 4, 256, 64, 64
N = H * W            # 4096 keys per batch
CK = 32              # q/k projection dim
NH = N // 2          # 2048 query rows per core
N_CORES = 8
NCH = NH // 512      # 4 n-chunks of 512 per core
MT = N // 128        # 32 m-tiles of 128


def _build():
    nc = bacc.Bacc("TRN2", target_bir_lowering=False, debug=False)
    xa = nc.dram_tensor("xa", [C, NH], F32R, kind="ExternalInput").ap()
    xb = nc.dram_tensor("xb", [C, N], F32R, kind="ExternalInput").ap()
    wqt = nc.dram_tensor("wqt", [C, CK], F32R, kind="ExternalInput").ap()
    wkt = nc.dram_tensor("wkt", [C, CK], F32R, kind="ExternalInput").ap()
    wvt = nc.dram_tensor("wvt", [C, C], F32R, kind="ExternalInput").ap()
    bq = nc.dram_tensor("bq", [CK, 1], F32, kind="ExternalInput").ap()
    bk = nc.dram_tensor("bk", [CK, 1], F32, kind="ExternalInput").ap()
    bva = nc.dram_tensor("bva", [1, 258], F32, kind="ExternalInput").ap()
    gam = nc.dram_tensor("gam", [1, 1], F32, kind="ExternalInput").ap()
    zed = nc.dram_tensor("zed", [1, N], F32R, kind="ExternalInput").ap()
    out = nc.dram_tensor("out", [C, NH], F32, kind="ExternalOutput").ap()

    with tile.TileContext(nc) as tc:
        with tc.tile_pool(name="const", bufs=1) as const, \
             tc.tile_pool(name="work", bufs=3) as work, \
             tc.tile_pool(name="outp", bufs=2) as outp, \
             tc.tile_pool(name="small", bufs=4) as small, \
             tc.tile_pool(name="stp", bufs=2, space="PSUM") as stp, \
             tc.tile_pool(name="opp", bufs=1, space="PSUM") as opp:

            # ---- constants / persistent tiles -------------------------
            xa_sb = [const.tile([128, NH], F32R, tag=f"xa{h}", name=f"xa_sb{h}") for h in range(2)]
            xa_f = [const.tile([128, NH], F32, tag=f"xaf{h}", name=f"xa_f{h}") for h in range(2)]
            xb_sb = [const.tile([128, N], F32R, tag=f"xb{h}", name=f"xb_sb{h}") for h in range(2)]
            wqt_sb = [const.tile([128, CK], F32R, tag=f"wq{h}", name=f"wqt_sb{h}") for h in range(2)]
            wkt_sb = [const.tile([128, CK], F32R, tag=f"wk{h}", name=f"wkt_sb{h}") for h in range(2)]
            wvt_sb = [const.tile([128, C], F32R, tag=f"wv{h}", name=f"wvt_sb{h}") for h in range(2)]
            CS = [slice(0, 128), slice(128, 256)]
            for h in range(2):
                nc.sync.dma_start(out=wqt_sb[h], in_=wqt[CS[h], :])
                nc.sync.dma_start(out=wkt_sb[h], in_=wkt[CS[h], :])
                nc.sync.dma_start(out=wvt_sb[h], in_=wvt[CS[h], :])
            # chunked loads, halves interleaved, so consumers start per slice
            for q in range(NH // 512):
                qs = slice(q * 512, (q + 1) * 512)
                for h in range(2):
                    nc.sync.dma_start(out=xa_sb[h][:, qs], in_=xa[CS[h], qs])
            for q in range(N // 512):
                qs = slice(q * 512, (q + 1) * 512)
                for h in range(2):
                    nc.sync.dma_start(out=xb_sb[h][:, qs], in_=xb[CS[h], qs])
            for q in range(NH // 512):
                qs = slice(q * 512, (q + 1) * 512)
                for h in range(2):
                    nc.sync.dma_start(out=xa_f[h][:, qs],
                                      in_=xa[CS[h], qs].bitcast(F32))
            bq_sb = const.tile([CK, 1], F32, tag="bq")
            bk_sb = const.tile([CK, 1], F32, tag="bk")
            nc.sync.dma_start(out=bq_sb, in_=bq)
            nc.sync.dma_start(out=bk_sb, in_=bk)
            bva_sb = const.tile([128, 258], F32, tag="bva")
            nc.sync.dma_start(out=bva_sb, in_=bva.to_broadcast((128, 258)))
            gam_sb = const.tile([128, 1], F32, tag="gam")
            nc.sync.dma_start(out=gam_sb, in_=gam.to_broadcast((128, 1)))
            ident = const.tile([128, 128], F32, tag="ident")
            make_identity(nc, ident)

            kmat = const.tile([128, N], F32R, tag="kmat")    # K=128-padded
            qtp = const.tile([128, NH], F32R, tag="qtp")     # K=128-padded
            nc.sync.dma_start(out=kmat[CK:128, :],
                              in_=zed.to_broadcast((128 - CK, N)))
            nc.sync.dma_start(out=qtp[CK:128, :],
                              in_=zed[:, 0:NH].to_broadcast((128 - CK, NH)))
            v_aug = const.tile([128, MT, 258], F32R, tag="vaug")

            # ---- projections -----------------------------------------
            # qT[o, n] = sum_c Wq[o, c] xa[c, n] ; psum [32, 512] per chunk
            for ch in range(NCH):
                ns = slice(ch * 512, (ch + 1) * 512)
                ps = stp.tile([CK, 512], F32, tag="st")
                for h in range(2):
                    nc.tensor.matmul(ps, wqt_sb[h], xa_sb[h][:, ns],
                                     start=(h == 0), stop=(h == 1))
                nc.vector.tensor_scalar_add(qtp[0:CK, ns], ps, bq_sb)
            for mch in range(N // 512):
                ms = slice(mch * 512, (mch + 1) * 512)
                ps = stp.tile([CK, 512], F32, tag="st")
                for h in range(2):
                    nc.tensor.matmul(ps, wkt_sb[h], xb_sb[h][:, ms],
                                     start=(h == 0), stop=(h == 1))
                nc.vector.tensor_scalar_add(kmat[0:CK, ms], ps, bk_sb)
            # v[m, c] = sum_cc xb[cc, m] Wv[c, cc] ; lhsT = xb tile slice.
            # Production is interleaved into n-chunk 0 of the main loop so
            # the PE never sits in a long v-only prep phase.
            def emit_v(i):
                ms = slice(i * 128, (i + 1) * 128)
                ps = stp.tile([128, C], F32, tag="st", name=f"vps_{i}")
                for h in range(2):
                    nc.tensor.matmul(ps, xb_sb[h][:, ms], wvt_sb[h],
                                     start=(h == 0), stop=(h == 1))
                nc.vector.tensor_add(v_aug[:, i, 0:C], ps, bva_sb[:, 0:C])
                nc.vector.tensor_copy(v_aug[:, i, C:258], bva_sb[:, C:258])

            # ---- attention main loop ---------------------------------
            def emit_st(ch, p):
                # logits for m-pair p of n-chunk ch -> [128, 1024] psum
                ns = slice(ch * 512, (ch + 1) * 512)
                mA, mB = 2 * p, 2 * p + 1
                st = stp.tile([128, 1024], F32, tag="st", name=f"st_{ch}_{p}")
                nc.tensor.matmul(st[:, 0:512],
                                 kmat[:, mA * 128:(mA + 1) * 128],
                                 qtp[:, ns], start=True, stop=True)
                nc.tensor.matmul(st[:, 512:1024],
                                 kmat[:, mB * 128:(mB + 1) * 128],
                                 qtp[:, ns], start=True, stop=True)
                ex = work.tile([128, 1024], F32R, tag="exp",
                               name=f"ex_{ch}_{p}")
                nc.scalar.activation(out=ex, in_=st, func=EXP)
                return ex

            NP = MT // 2
            for ch in range(NCH):
                ops = [opp.tile([128, 258], F32, tag=f"out{j}", name=f"ops{j}")
                       for j in range(4)]
                if ch == 0:
                    ex_next = emit_st(0, 0)
                for p in range(NP):
                    mA, mB = 2 * p, 2 * p + 1
                    if ch == 0:
                        emit_v(mA)
                        emit_v(mB)
                    ex = ex_next
                    # issue next pair's ST/exp before this pair's out-MMs so
                    # ACT(exp) overlaps PE(out) instead of serializing
                    if p + 1 < NP:
                        ex_next = emit_st(ch, p + 1)
                    elif ch + 1 < NCH:
                        ex_next = emit_st(ch + 1, 0)
                    for j in range(4):
                        js = slice(j * 128, (j + 1) * 128)
                        nc.tensor.matmul(ops[j], ex[:, js], v_aug[:, mA, :],
                                         start=(p == 0), stop=False,
                                         skip_group_check=True)
                        js2 = slice(512 + j * 128, 512 + (j + 1) * 128)
                        nc.tensor.matmul(ops[j], ex[:, js2], v_aug[:, mB, :],
                                         start=False, stop=(p == NP - 1),
                                         skip_group_check=True)

                # ---- finalize this n-chunk ---------------------------
                otiles = [outp.tile([128, 512], F32, tag=f"ot{h}", name=f"otile{h}")
                          for h in range(2)]
                for j in range(4):
                    rsum = small.tile([128, 1], F32, tag="rsum")
                    nc.vector.reciprocal(rsum, ops[j][:, 256:257])
                    sc2 = small.tile([128, 1], F32, tag="sc2")
                    nc.vector.tensor_mul(sc2, rsum, gam_sb)
                    scaled = work.tile([128, C], F32, tag="scaled")
                    nc.vector.tensor_scalar_mul(scaled, ops[j][:, 0:C], sc2)
                    for h in range(2):
                        tp = stp.tile([128, 128], F32, tag="st")
                        nc.tensor.transpose(tp, scaled[:, h * 128:(h + 1) * 128],
                                            ident)
                        nc.vector.tensor_add(
                            otiles[h][:, j * 128:(j + 1) * 128], tp,
                            xa_f[h][:, ch * 512 + j * 128:
                                    ch * 512 + (j + 1) * 128])
                for h in range(2):
                    nc.sync.dma_start(
                        out=out[h * 128:(h + 1) * 128, ch * 512:(ch + 1) * 512],
                        in_=otiles[h])
    nc.compile()
    return nc


_NC_CACHE = None


def _get_nc():
    global _NC_CACHE
    if _NC_CACHE is None:
        _NC_CACHE = _build()
    return _NC_CACHE


def kernel(x_a, x_b, Wq, bq, Wk, bk, Wv, bv, gamma):
    x_a = np.ascontiguousarray(np.asarray(x_a, dtype=np.float32))
    x_b = np.ascontiguousarray(np.asarray(x_b, dtype=np.float32))
    Wq = np.asarray(Wq, dtype=np.float32)
    Wk = np.asarray(Wk, dtype=np.float32)
    Wv = np.asarray(Wv, dtype=np.float32)
    bqv = np.asarray(bq, dtype=np.float32).reshape(CK, 1)
    bkv = np.asarray(bk, dtype=np.float32).reshape(CK, 1)
    bvv = np.asarray(bv, dtype=np.float32)
    gv = np.asarray(gamma, dtype=np.float32).reshape(1, 1)

    xaf = x_a.reshape(B, C, N)
    xbf = x_b.reshape(B, C, N)
    wqt = np.ascontiguousarray(Wq.T)
    wkt = np.ascontiguousarray(Wk.T)
    wvt = np.ascontiguousarray(Wv.T)
    bva = np.concatenate([bvv, np.array([1.0, 0.0], np.float32)]).reshape(1, 258)

    in_maps = []
    for c in range(N_CORES):
        b, half = c // 2, c % 2
        in_maps.append({
            "xa": np.ascontiguousarray(xaf[b, :, half * NH:(half + 1) * NH]),
            "xb": np.ascontiguousarray(xbf[b]),
            "wqt": wqt, "wkt": wkt, "wvt": wvt,
            "bq": bqv, "bk": bkv, "bva": bva, "gam": gv,
            "zed": np.zeros((1, N), np.float32),
        })

    nc = _get_nc()
    res = bass_utils.run_bass_kernel_spmd(nc, in_maps,
                                          core_ids=list(range(N_CORES)))
    out = np.empty((B, C, N), np.float32)
    for c in range(N_CORES):
        b, half = c // 2, c % 2
        out[b, :, half * NH:(half + 1) * NH] = res.results[c]["out"]
    return out.reshape(B, C, H, W)
